# revision 5
# baseline (speedup 1.0000x reference)
"""FFTBlock (attention + conv-FFN transformer block) on 8 Trainium2 NeuronCores.

Data-parallel over batch: 16 batch items -> 2 per core. Each core runs the
full block (MHA + LN + conv1d-FFN + LN) on its 2 batch items.

Layout strategy (per batch item, per core):
  - Host pre-transposes x -> xT [D,S] (bf16) so QKV projections contract D on
    partitions; host pre-transposes mask -> maskT [S_k,S_q] (uint8).
  - Q,K produced transposed (QT/KT [DH,S]); V natural [S,DH] with a ones
    column appended so the A@V matmul also produces softmax denominators.
  - Scores computed transposed (scoresT [S_k,S_q]); softmax is exp-only
    (no max subtraction needed: scores are O(1); masked lanes underflow to 0
    exactly, matching the reference), denominator comes from the ones column.
  - Out-projection consumes the normalized O-transposed chunks directly and
    lands attention output in natural [S,D] layout for a free-dim layernorm.
  - conv1 produces hT [F,S] (weights stationary), conv2 consumes hT slices as
    stationary operands and lands natural [S,D] for the second layernorm.
  - Both convs implement K=9 'same' padding via shifted slices of a
    zero-padded S axis (4+S+4 columns).
"""

import sys

sys.path.insert(0, "/opt/trn_rl_repo")

import math

import ml_dtypes
import numpy as np

import concourse.bass as bass
import concourse.mybir as mybir
import concourse.tile as tile
from concourse import bacc
from concourse.bass_utils import run_bass_kernel_spmd
from concourse.masks import make_identity

BF16 = mybir.dt.bfloat16
F32 = mybir.dt.float32
U8 = mybir.dt.uint8

B, S, D, H, DH, F, K = 16, 1024, 384, 2, 192, 1536, 9
NCORES = 8
NB = B // NCORES  # batch items per core
EPS = 1e-5
ISCALE = 1.0 / math.sqrt(D)  # NOTE: reference scales by sqrt(d_model)
SP = S + 8  # padded sequence length (4 left, 4 right)
DC = D // 128  # 3 d-chunks
FT = F // 128  # 12 filter tiles
ST = S // 128  # 8 seq tiles of 128
SQ = S // 512  # 2 seq chunks of 512
NEG = -1.0e9

_CACHE = {}


def _emit(nc):
    # ---- DRAM I/O ----
    xT_d = nc.dram_tensor("xT", [NB, DC, 128, S], BF16, kind="ExternalInput")
    xn_d = nc.dram_tensor("xn", [NB, ST, 128, D], F32, kind="ExternalInput")
    mT_d = nc.dram_tensor("mT", [NB, ST, 128, S], U8, kind="ExternalInput")
    wq_d = nc.dram_tensor("wq", [H, DC, 128, DH], BF16, kind="ExternalInput")
    wk_d = nc.dram_tensor("wk", [H, DC, 128, DH], BF16, kind="ExternalInput")
    wv_d = nc.dram_tensor("wv", [H, DC, 128, DH], BF16, kind="ExternalInput")
    wo_d = nc.dram_tensor("wo", [4, 128, D], BF16, kind="ExternalInput")
    wc1_d = nc.dram_tensor("wc1", [K, DC, 128, F], BF16, kind="ExternalInput")
    wc2_d = nc.dram_tensor("wc2", [K, FT, 128, D], BF16, kind="ExternalInput")
    bqk_d = nc.dram_tensor("bqk", [2, H, 2, 128], F32, kind="ExternalInput")
    bv_d = nc.dram_tensor("bv", [H, DH], F32, kind="ExternalInput")
    bo_d = nc.dram_tensor("bo", [D], F32, kind="ExternalInput")
    bc1_d = nc.dram_tensor("bc1t", [128, FT], F32, kind="ExternalInput")
    bc2_d = nc.dram_tensor("bc2", [D], F32, kind="ExternalInput")
    g1_d = nc.dram_tensor("g1", [D], F32, kind="ExternalInput")
    be1_d = nc.dram_tensor("be1", [D], F32, kind="ExternalInput")
    g2_d = nc.dram_tensor("g2", [D], F32, kind="ExternalInput")
    be2_d = nc.dram_tensor("be2", [D], F32, kind="ExternalInput")
    y_d = nc.dram_tensor("y", [NB, ST, 128, D], F32, kind="ExternalOutput")

    with tile.TileContext(nc) as tc:
        _body(nc, tc, locals())
    nc.finalize()
    return nc


def _bcast(ap, p=128):
    return bass.AP(tensor=ap.tensor, offset=ap.offset, ap=[[0, p]] + list(ap.ap))


def _body(nc, tc, d):
    xT_d, xn_d, mT_d = d["xT_d"], d["xn_d"], d["mT_d"]
    wq_d, wk_d, wv_d, wo_d = d["wq_d"], d["wk_d"], d["wv_d"], d["wo_d"]
    wc1_d, wc2_d = d["wc1_d"], d["wc2_d"]
    bqk_d, bv_d, bo_d, bc1_d, bc2_d = (
        d["bqk_d"], d["bv_d"], d["bo_d"], d["bc1_d"], d["bc2_d"],
    )
    g1_d, be1_d, g2_d, be2_d, y_d = d["g1_d"], d["be1_d"], d["g2_d"], d["be2_d"], d["y_d"]

    from contextlib import ExitStack

    ctx = ExitStack()
    with ctx:
        const = ctx.enter_context(tc.tile_pool(name="const", bufs=1))
        persist = ctx.enter_context(tc.tile_pool(name="persist", bufs=1))

        # ---- constants / weights resident in SBUF ----
        ident = const.tile([128, 128], F32, tag="ident")
        make_identity(nc, ident[:])

        wq_sb = const.tile([128, H, DC, DH], BF16, tag="wq")
        nc.sync.dma_start(wq_sb[:], wq_d[:].rearrange("h c p e -> p h c e"))
        wk_sb = const.tile([128, H, DC, DH], BF16, tag="wk")
        nc.sync.dma_start(wk_sb[:], wk_d[:].rearrange("h c p e -> p h c e"))
        wv_sb = const.tile([128, H, DC, DH], BF16, tag="wv")
        nc.sync.dma_start(wv_sb[:], wv_d[:].rearrange("h c p e -> p h c e"))
        wo_sb = const.tile([128, 4, D], BF16, tag="wo")
        nc.sync.dma_start(wo_sb[:], wo_d[:].rearrange("c p e -> p c e"))

        bqk_sb = const.tile([128, 2, H, 2], F32, tag="bqk")
        nc.sync.dma_start(bqk_sb[:], bqk_d[:].rearrange("a h c p -> p a h c"))
        bv_sb = const.tile([128, H, DH], F32, tag="bv")
        nc.sync.dma_start(bv_sb[:], _bcast(bv_d[:]))
        bo_sb = const.tile([128, D], F32, tag="bo")
        nc.sync.dma_start(bo_sb[:], _bcast(bo_d[:]))
        bc1_sb = const.tile([128, FT], F32, tag="bc1")
        nc.sync.dma_start(bc1_sb[:], bc1_d[:])
        bc2_sb = const.tile([128, D], F32, tag="bc2")
        nc.sync.dma_start(bc2_sb[:], _bcast(bc2_d[:]))
        g1_sb = const.tile([128, D], F32, tag="g1")
        nc.sync.dma_start(g1_sb[:], _bcast(g1_d[:]))
        be1_sb = const.tile([128, D], F32, tag="be1")
        nc.sync.dma_start(be1_sb[:], _bcast(be1_d[:]))
        g2_sb = const.tile([128, D], F32, tag="g2")
        nc.sync.dma_start(g2_sb[:], _bcast(g2_d[:]))
        be2_sb = const.tile([128, D], F32, tag="be2")
        nc.sync.dma_start(be2_sb[:], _bcast(be2_d[:]))

        eps_sb = const.tile([128, 1], F32, tag="eps")
        nc.vector.memset(eps_sb[:], EPS)

        # ---- persistent activations ----
        x1T = persist.tile([128, NB, DC, SP], BF16, tag="x1T")  # LN1 out, transposed+padded
        nc.vector.memset(x1T[:], 0.0)
        x1n = persist.tile([128, NB, ST, D], F32, tag="x1n")  # LN1 out, natural
        hT = persist.tile([128, NB, FT, SP], BF16, tag="hT")  # conv1 out, transposed+padded
        nc.vector.memset(hT[:], 0.0)

        # ================= attention (per batch item) =================
        for b in range(NB):
            with ExitStack() as bctx:
                asb = bctx.enter_context(tc.tile_pool(name=f"asb{b}", bufs=1))
                expp = bctx.enter_context(tc.tile_pool(name=f"exp{b}", bufs=2))
                lnp = bctx.enter_context(tc.tile_pool(name=f"ln{b}", bufs=3))
                smal = bctx.enter_context(tc.tile_pool(name=f"sm{b}", bufs=2))

                xT_sb = asb.tile([128, DC, S], BF16, tag="xT")
                nc.sync.dma_start(xT_sb[:], xT_d[b].rearrange("c p s -> p c s"))
                mT_sb = asb.tile([128, ST, S], U8, tag="mT")
                nc.sync.dma_start(mT_sb[:], mT_d[b].rearrange("c p q -> p c q"))

                qt, kt, vv = [], [], []
                with tc.tile_pool(name=f"psA{b}", bufs=2, space="PSUM") as psA:
                    for h in range(H):
                        qt.append(asb.tile([128, 2, S], BF16, name=f"qt{h}", tag=f"qt{h}"))
                        kt.append(asb.tile([128, 2, S], BF16, name=f"kt{h}", tag=f"kt{h}"))
                        vv.append(asb.tile([128, ST, DH + 1], BF16, name=f"vv{h}", tag=f"vv{h}"))
                        for wsb, bi, dst in ((wq_sb, 0, qt[h]), (wk_sb, 1, kt[h])):
                            for mc, (m0, msz) in enumerate(((0, 128), (128, 64))):
                                for qc in range(SQ):
                                    ps = psA.tile([128, 512], F32, tag="qk")
                                    for dc in range(DC):
                                        nc.tensor.matmul(
                                            ps[:msz, :],
                                            lhsT=wsb[:, h, dc, m0 : m0 + msz],
                                            rhs=xT_sb[:, dc, qc * 512 : qc * 512 + 512],
                                            start=(dc == 0),
                                            stop=(dc == DC - 1),
                                        )
                                    nc.scalar.activation(
                                        out=dst[:msz, mc, qc * 512 : qc * 512 + 512],
                                        in_=ps[:msz, :],
                                        func=mybir.ActivationFunctionType.Identity,
                                        bias=bqk_sb[:msz, bi, h, mc : mc + 1],
                                        scale=1.0,
                                    )
                        for st in range(ST):
                            ps = psA.tile([128, DH], F32, tag="v")
                            for dc in range(DC):
                                nc.tensor.matmul(
                                    ps[:],
                                    lhsT=xT_sb[:, dc, st * 128 : st * 128 + 128],
                                    rhs=wv_sb[:, h, dc, :],
                                    start=(dc == 0),
                                    stop=(dc == DC - 1),
                                )
                            nc.vector.tensor_add(
                                out=vv[h][:, st, 0:DH], in0=ps[:], in1=bv_sb[:, h, :]
                            )
                            nc.vector.memset(vv[h][:, st, DH : DH + 1], 1.0)

                # scores -> exp -> O, per head
                onrm = []
                for h in range(H):
                    expT = expp.tile([128, ST, S], BF16, tag="expT")
                    with tc.tile_pool(name=f"psB{b}_{h}", bufs=3, space="PSUM") as psB:
                        for kc in range(ST):
                            for qc in range(SQ):
                                qs = slice(qc * 512, qc * 512 + 512)
                                ps = psB.tile([128, 512], F32, tag="sc")
                                for mc, (m0, msz) in enumerate(((0, 128), (128, 64))):
                                    nc.tensor.matmul(
                                        ps[:],
                                        lhsT=kt[h][:msz, mc, kc * 128 : kc * 128 + 128],
                                        rhs=qt[h][:msz, mc, qs],
                                        start=(mc == 0),
                                        stop=(mc == 1),
                                    )
                                nc.vector.scalar_tensor_tensor(
                                    out=ps[:],
                                    in0=mT_sb[:, kc, qs],
                                    scalar=NEG,
                                    in1=ps[:],
                                    op0=mybir.AluOpType.mult,
                                    op1=mybir.AluOpType.add,
                                )
                                nc.scalar.activation(
                                    out=expT[:, kc, qs],
                                    in_=ps[:],
                                    func=mybir.ActivationFunctionType.Exp,
                                    scale=ISCALE,
                                )
                    onrm.append(asb.tile([128, 2, S], BF16, name=f"on{h}", tag=f"on{h}"))
                    with tc.tile_pool(name=f"psC{b}_{h}", bufs=1, space="PSUM") as psC:
                        for qc in range(SQ):
                            qs = slice(qc * 512, qc * 512 + 512)
                            ps0 = psC.tile([128, 512], F32, tag="ot0")
                            ps1 = psC.tile([65, 512], F32, tag="ot1")
                            for kc in range(ST):
                                nc.tensor.matmul(
                                    ps0[:],
                                    lhsT=vv[h][:, kc, 0:128],
                                    rhs=expT[:, kc, qs],
                                    start=(kc == 0),
                                    stop=(kc == ST - 1),
                                )
                                nc.tensor.matmul(
                                    ps1[:],
                                    lhsT=vv[h][:, kc, 128 : DH + 1],
                                    rhs=expT[:, kc, qs],
                                    start=(kc == 0),
                                    stop=(kc == ST - 1),
                                )
                            rc = smal.tile([1, 512], F32, tag="rc")
                            nc.vector.reciprocal(rc[:], ps1[64:65, :])
                            rb = smal.tile([128, 512], F32, tag="rb")
                            nc.gpsimd.partition_broadcast(rb[:], rc[:])
                            nc.vector.tensor_mul(
                                out=onrm[h][:, 0, qs], in0=ps0[:], in1=rb[:]
                            )
                            nc.vector.tensor_mul(
                                out=onrm[h][:64, 1, qs], in0=ps1[:64, :], in1=rb[:64, :]
                            )

                # out-projection + residual + LN1 + transpose to x1T
                chunks = ((0, 0, 128, 0), (0, 1, 64, 1), (1, 0, 128, 2), (1, 1, 64, 3))
                with (
                    tc.tile_pool(name=f"psD{b}", bufs=2, space="PSUM") as psD,
                    tc.tile_pool(name=f"psE{b}", bufs=2, space="PSUM") as psE,
                ):
                    for st in range(ST):
                        ps = psD.tile([128, D], F32, tag="at")
                        for i, (h, c, ksz, wc) in enumerate(chunks):
                            nc.tensor.matmul(
                                ps[:],
                                lhsT=onrm[h][:ksz, c, st * 128 : st * 128 + 128],
                                rhs=wo_sb[:ksz, wc, :],
                                start=(i == 0),
                                stop=(i == 3),
                            )
                        xn_t = lnp.tile([128, D], F32, tag="xn")
                        nc.sync.dma_start(xn_t[:], xn_d[b, st])
                        t = lnp.tile([128, D], F32, tag="t")
                        nc.vector.tensor_add(out=t[:], in0=ps[:], in1=xn_t[:])
                        nc.vector.tensor_add(out=t[:], in0=t[:], in1=bo_sb[:])
                        stats = lnp.tile([128, 6], F32, tag="st")
                        nc.vector.bn_stats(out=stats[:], in_=t[:])
                        mv = lnp.tile([128, 2], F32, tag="mv")
                        nc.vector.bn_aggr(out=mv[:], in_=stats[:])
                        sd = lnp.tile([128, 1], F32, tag="sd")
                        nc.scalar.activation(
                            out=sd[:], in_=mv[:, 1:2],
                            func=mybir.ActivationFunctionType.Sqrt, bias=eps_sb[:],
                        )
                        nc.vector.reciprocal(sd[:], sd[:])
                        xv = x1n[:, b, st, :]
                        nc.vector.tensor_scalar(
                            out=xv, in0=t[:], scalar1=mv[:, 0:1], scalar2=sd[:],
                            op0=mybir.AluOpType.subtract, op1=mybir.AluOpType.mult,
                        )
                        nc.vector.tensor_mul(out=xv, in0=xv, in1=g1_sb[:])
                        nc.vector.tensor_add(out=xv, in0=xv, in1=be1_sb[:])
                        for dc in range(DC):
                            tp = psE.tile([128, 128], F32, tag="tp")
                            nc.tensor.transpose(
                                tp[:], x1n[:, b, st, dc * 128 : dc * 128 + 128], ident[:]
                            )
                            nc.vector.tensor_copy(
                                out=x1T[:, b, dc, 4 + st * 128 : 4 + st * 128 + 128],
                                in_=tp[:],
                            )

        # ================= conv1: x1T -> hT (relu) =================
        with (
            tc.tile_pool(name="w1p", bufs=2) as w1p,
            tc.tile_pool(name="psF", bufs=4, space="PSUM") as psF,
        ):
            for ft in range(FT):
                w1 = w1p.tile([128, K, DC, 128], BF16, tag="w1")
                nc.sync.dma_start(
                    w1[:],
                    wc1_d[:, :, :, ft * 128 : ft * 128 + 128].rearrange(
                        "k c p f -> p k c f"
                    ),
                )
                for b in range(NB):
                    for qc in range(SQ):
                        ps = psF.tile([128, 512], F32, tag="c1")
                        idx = 0
                        for k9 in range(K):
                            for dc in range(DC):
                                nc.tensor.matmul(
                                    ps[:],
                                    lhsT=w1[:, k9, dc, :],
                                    rhs=x1T[:, b, dc, qc * 512 + k9 : qc * 512 + k9 + 512],
                                    start=(idx == 0),
                                    stop=(idx == K * DC - 1),
                                )
                                idx += 1
                        nc.scalar.activation(
                            out=hT[:, b, ft, 4 + qc * 512 : 4 + qc * 512 + 512],
                            in_=ps[:],
                            func=mybir.ActivationFunctionType.Relu,
                            bias=bc1_sb[:, ft : ft + 1],
                            scale=1.0,
                        )

        # ================= conv2 + residual + LN2 -> y =================
        with (
            tc.tile_pool(name="w2p", bufs=1) as w2p,
            tc.tile_pool(name="psG", bufs=4, space="PSUM") as psG,
            tc.tile_pool(name="ln2", bufs=3) as ln2,
        ):
            w2 = w2p.tile([128, K, FT, D], BF16, tag="w2")
            nc.sync.dma_start(w2[:], wc2_d[:].rearrange("k c p e -> p k c e"))
            for b in range(NB):
                for st in range(ST):
                    ps = psG.tile([128, D], F32, tag="c2")
                    idx = 0
                    for k9 in range(K):
                        for fc in range(FT):
                            nc.tensor.matmul(
                                ps[:],
                                lhsT=hT[:, b, fc, st * 128 + k9 : st * 128 + k9 + 128],
                                rhs=w2[:, k9, fc, :],
                                start=(idx == 0),
                                stop=(idx == K * FT - 1),
                            )
                            idx += 1
                    t = ln2.tile([128, D], F32, tag="t")
                    nc.vector.tensor_add(out=t[:], in0=ps[:], in1=x1n[:, b, st, :])
                    nc.vector.tensor_add(out=t[:], in0=t[:], in1=bc2_sb[:])
                    stats = ln2.tile([128, 6], F32, tag="st")
                    nc.vector.bn_stats(out=stats[:], in_=t[:])
                    mv = ln2.tile([128, 2], F32, tag="mv")
                    nc.vector.bn_aggr(out=mv[:], in_=stats[:])
                    sd = ln2.tile([128, 1], F32, tag="sd")
                    nc.scalar.activation(
                        out=sd[:], in_=mv[:, 1:2],
                        func=mybir.ActivationFunctionType.Sqrt, bias=eps_sb[:],
                    )
                    nc.vector.reciprocal(sd[:], sd[:])
                    ot = ln2.tile([128, D], F32, tag="o")
                    nc.vector.tensor_scalar(
                        out=ot[:], in0=t[:], scalar1=mv[:, 0:1], scalar2=sd[:],
                        op0=mybir.AluOpType.subtract, op1=mybir.AluOpType.mult,
                    )
                    nc.vector.tensor_mul(out=ot[:], in0=ot[:], in1=g2_sb[:])
                    nc.vector.tensor_add(out=ot[:], in0=ot[:], in1=be2_sb[:])
                    nc.sync.dma_start(y_d[b, st], ot[:])


def _build():
    if "nc" not in _CACHE:
        nc = bacc.Bacc()
        _CACHE["nc"] = _emit(nc)
    return _CACHE["nc"]


def _prep_shared(Wq, bq, Wk, bk, Wv, bv, Wo, bo, Wc1, bc1, Wc2, bc2, g1, beta1, g2, beta2):
    bf = ml_dtypes.bfloat16
    f32 = np.float32
    sh = {}
    sh["wq"] = np.ascontiguousarray(Wq.reshape(H, DC, 128, DH).astype(bf))
    sh["wk"] = np.ascontiguousarray(Wk.reshape(H, DC, 128, DH).astype(bf))
    sh["wv"] = np.ascontiguousarray(Wv.reshape(H, DC, 128, DH).astype(bf))
    wo = np.zeros((4, 128, D), dtype=bf)
    bounds = ((0, 128), (128, 192), (192, 320), (320, 384))
    for c, (r0, r1) in enumerate(bounds):
        wo[c, : r1 - r0] = Wo[r0:r1].astype(bf)
    sh["wo"] = wo
    sh["wc1"] = np.ascontiguousarray(Wc1.reshape(K, DC, 128, F).astype(bf))
    sh["wc2"] = np.ascontiguousarray(Wc2.reshape(K, FT, 128, D).astype(bf))
    bqk = np.zeros((2, H, 2, 128), dtype=f32)
    for i, bb in enumerate((bq, bk)):
        for h in range(H):
            bqk[i, h, 0, :] = bb[h, :128]
            bqk[i, h, 1, :64] = bb[h, 128:]
    sh["bqk"] = bqk
    sh["bv"] = bv.astype(f32)
    sh["bo"] = bo.astype(f32)
    sh["bc1t"] = np.ascontiguousarray(bc1.reshape(FT, 128).T.astype(f32))
    sh["bc2"] = bc2.astype(f32)
    sh["g1"] = g1.astype(f32)
    sh["be1"] = beta1.astype(f32)
    sh["g2"] = g2.astype(f32)
    sh["be2"] = beta2.astype(f32)
    return sh


def run_sharded(inputs, trace=False):
    nc = _build()
    x = np.asarray(inputs["x"], dtype=np.float32)
    mask = np.asarray(inputs["mask"])
    sh = _prep_shared(
        *[np.asarray(inputs[k]) for k in (
            "Wq", "bq", "Wk", "bk", "Wv", "bv", "Wo", "bo",
            "Wc1", "bc1", "Wc2", "bc2", "g1", "beta1", "g2", "beta2",
        )]
    )
    bf = ml_dtypes.bfloat16
    in_maps = []
    for c in range(NCORES):
        xb = x[c * NB : (c + 1) * NB]  # [NB, S, D]
        m = {}
        m["xT"] = np.ascontiguousarray(xb.transpose(0, 2, 1)).reshape(NB, DC, 128, S).astype(bf)
        m["xn"] = np.ascontiguousarray(xb.reshape(NB, ST, 128, D))
        mb = mask[c * NB : (c + 1) * NB]
        m["mT"] = np.ascontiguousarray(mb.transpose(0, 2, 1)).astype(np.uint8).reshape(NB, ST, 128, S)
        m.update(sh)
        in_maps.append(m)
    res = run_bass_kernel_spmd(nc, in_maps, core_ids=list(range(NCORES)), trace=trace)
    out = np.empty((B, S, D), dtype=np.float32)
    for c in range(NCORES):
        out[c * NB : (c + 1) * NB] = res.results[c]["y"].reshape(NB, S, D)
    return out, res


def kernel(**inputs):
    out, _ = run_sharded(inputs, trace=False)
    return out


# revision 6
# speedup vs baseline: 1.1441x; 1.1441x over previous
"""FFTBlock (attention + conv-FFN transformer block) on 8 Trainium2 NeuronCores.

Data-parallel over batch: 16 batch items -> 2 per core. Each core runs the
full block (MHA + LN + conv1d-FFN + LN) on its 2 batch items.

Layout strategy (per batch item, per core):
  - Host pre-transposes x -> xT [D,S] (bf16) so QKV projections contract D on
    partitions; host pre-transposes mask -> maskT [S_k,S_q] (uint8).
  - Q,K produced transposed (QT/KT [DH,S]); V natural [S,DH] with a ones
    column appended so the A@V matmul also produces softmax denominators.
  - Scores computed transposed (scoresT [S_k,S_q]); softmax is exp-only
    (no max subtraction needed: scores are O(1); masked lanes underflow to 0
    exactly, matching the reference), denominator comes from the ones column.
  - Out-projection consumes the normalized O-transposed chunks directly and
    lands attention output in natural [S,D] layout for a free-dim layernorm.
  - conv1 produces hT [F,S] (weights stationary), conv2 consumes hT slices as
    stationary operands and lands natural [S,D] for the second layernorm.
  - Both convs implement K=9 'same' padding via shifted slices of a
    zero-padded S axis (4+S+4 columns).
"""

import sys

sys.path.insert(0, "/opt/trn_rl_repo")

import math
from contextlib import ExitStack

import ml_dtypes
import numpy as np

import concourse.bass as bass
import concourse.mybir as mybir
import concourse.tile as tile
from concourse import bacc
from concourse.bass_utils import run_bass_kernel_spmd
from concourse.masks import make_identity

BF16 = mybir.dt.bfloat16
F32 = mybir.dt.float32
U8 = mybir.dt.uint8
AF = mybir.ActivationFunctionType
ALU = mybir.AluOpType

B, S, D, H, DH, F, K = 16, 1024, 384, 2, 192, 1536, 9
NCORES = 8
NB = B // NCORES  # batch items per core
EPS = 1e-5
ISCALE = 1.0 / math.sqrt(D)  # NOTE: reference scales by sqrt(d_model)
SP = S + 8  # padded sequence length (4 left, 4 right)
DC = D // 128  # 3 d-chunks
FT = F // 128  # 12 filter tiles
ST = S // 128  # 8 seq tiles of 128
SQ = S // 512  # 2 seq chunks of 512
NEG = -1.0e9

_CACHE = {}


def _bcast(ap, p=128):
    return bass.AP(tensor=ap.tensor, offset=ap.offset, ap=[[0, p]] + list(ap.ap))


def _emit(nc):
    # ---- DRAM I/O ----
    xT_d = nc.dram_tensor("xT", [NB, DC, 128, S], BF16, kind="ExternalInput")
    xn_d = nc.dram_tensor("xn", [NB, ST, 128, D], F32, kind="ExternalInput")
    mT_d = nc.dram_tensor("mT", [NB, ST, 128, S], U8, kind="ExternalInput")
    wq_d = nc.dram_tensor("wq", [H, DC, 128, DH], BF16, kind="ExternalInput")
    wk_d = nc.dram_tensor("wk", [H, DC, 128, DH], BF16, kind="ExternalInput")
    wv_d = nc.dram_tensor("wv", [H, DC, 128, DH], BF16, kind="ExternalInput")
    wo_d = nc.dram_tensor("wo", [4, 128, D], BF16, kind="ExternalInput")
    wc1_d = nc.dram_tensor("wc1", [K, DC, 128, F], BF16, kind="ExternalInput")
    wc2_d = nc.dram_tensor("wc2", [K, FT, 128, D], BF16, kind="ExternalInput")
    bqk_d = nc.dram_tensor("bqk", [2, H, 2, 128], F32, kind="ExternalInput")
    bv_d = nc.dram_tensor("bv", [H, DH], F32, kind="ExternalInput")
    bo_d = nc.dram_tensor("bo", [D], F32, kind="ExternalInput")
    bc1_d = nc.dram_tensor("bc1t", [128, FT], F32, kind="ExternalInput")
    bc2_d = nc.dram_tensor("bc2", [D], F32, kind="ExternalInput")
    g1_d = nc.dram_tensor("g1", [D], F32, kind="ExternalInput")
    be1_d = nc.dram_tensor("be1", [D], F32, kind="ExternalInput")
    g2_d = nc.dram_tensor("g2", [D], F32, kind="ExternalInput")
    be2_d = nc.dram_tensor("be2", [D], F32, kind="ExternalInput")
    y_d = nc.dram_tensor("y", [NB, ST, 128, D], F32, kind="ExternalOutput")

    with tile.TileContext(nc) as tc:
        _body(nc, tc, locals())
    nc.finalize()
    return nc


def _body(nc, tc, d):
    xT_d, xn_d, mT_d = d["xT_d"], d["xn_d"], d["mT_d"]
    wq_d, wk_d, wv_d, wo_d = d["wq_d"], d["wk_d"], d["wv_d"], d["wo_d"]
    wc1_d, wc2_d = d["wc1_d"], d["wc2_d"]
    bqk_d, bv_d, bo_d, bc1_d, bc2_d = (
        d["bqk_d"], d["bv_d"], d["bo_d"], d["bc1_d"], d["bc2_d"],
    )
    g1_d, be1_d, g2_d, be2_d, y_d = d["g1_d"], d["be1_d"], d["g2_d"], d["be2_d"], d["y_d"]

    with ExitStack() as ctx:
        const = ctx.enter_context(tc.tile_pool(name="const", bufs=1))
        persist = ctx.enter_context(tc.tile_pool(name="persist", bufs=1))

        # ---- critical-path weights first ----
        wq_sb = const.tile([128, H, DC, DH], BF16, tag="wq")
        nc.sync.dma_start(wq_sb[:], wq_d[:].rearrange("h c p e -> p h c e"))
        wk_sb = const.tile([128, H, DC, DH], BF16, tag="wk")
        nc.sync.dma_start(wk_sb[:], wk_d[:].rearrange("h c p e -> p h c e"))
        wv_sb = const.tile([128, H, DC, DH], BF16, tag="wv")
        nc.sync.dma_start(wv_sb[:], wv_d[:].rearrange("h c p e -> p h c e"))
        bqk_sb = const.tile([128, 2, H, 2], F32, tag="bqk")
        nc.sync.dma_start(bqk_sb[:], bqk_d[:].rearrange("a h c p -> p a h c"))
        bv_sb = const.tile([128, H, DH], F32, tag="bv")
        nc.sync.dma_start(bv_sb[:], _bcast(bv_d[:]))

        # ---- attention section ----
        with ExitStack() as actx:
            attn = actx.enter_context(tc.tile_pool(name="attn", bufs=1))
            expp = actx.enter_context(tc.tile_pool(name="expp", bufs=2))
            mskp = actx.enter_context(tc.tile_pool(name="mskp", bufs=4))
            lnp = actx.enter_context(tc.tile_pool(name="lnp", bufs=3))
            smal = actx.enter_context(tc.tile_pool(name="smal", bufs=2))
            psA = actx.enter_context(tc.tile_pool(name="psA", bufs=2, space="PSUM"))
            psB = actx.enter_context(tc.tile_pool(name="psB", bufs=2, space="PSUM"))
            psC = actx.enter_context(tc.tile_pool(name="psC", bufs=1, space="PSUM"))
            psD = actx.enter_context(tc.tile_pool(name="psD", bufs=1, space="PSUM"))
            psE = actx.enter_context(tc.tile_pool(name="psE", bufs=1, space="PSUM"))

            xT_sb = [
                attn.tile([128, DC, S], BF16, name=f"xT{b}", tag=f"xT{b}")
                for b in range(NB)
            ]
            for b in range(NB):
                nc.sync.dma_start(xT_sb[b][:], xT_d[b].rearrange("c p s -> p c s"))

            # remaining constants (off the critical path)
            wo_sb = const.tile([128, 4, D], BF16, tag="wo")
            nc.sync.dma_start(wo_sb[:], wo_d[:].rearrange("c p e -> p c e"))
            ident = const.tile([128, 128], F32, tag="ident")
            make_identity(nc, ident[:])
            bo_sb = const.tile([128, D], F32, tag="bo")
            nc.sync.dma_start(bo_sb[:], _bcast(bo_d[:]))
            bc1_sb = const.tile([128, FT], F32, tag="bc1")
            nc.sync.dma_start(bc1_sb[:], bc1_d[:])
            bc2_sb = const.tile([128, D], F32, tag="bc2")
            nc.sync.dma_start(bc2_sb[:], _bcast(bc2_d[:]))
            g1_sb = const.tile([128, D], F32, tag="g1")
            nc.sync.dma_start(g1_sb[:], _bcast(g1_d[:]))
            be1_sb = const.tile([128, D], F32, tag="be1")
            nc.sync.dma_start(be1_sb[:], _bcast(be1_d[:]))
            g2_sb = const.tile([128, D], F32, tag="g2")
            nc.sync.dma_start(g2_sb[:], _bcast(g2_d[:]))
            be2_sb = const.tile([128, D], F32, tag="be2")
            nc.sync.dma_start(be2_sb[:], _bcast(be2_d[:]))
            eps_sb = const.tile([128, 1], F32, tag="eps")
            nc.vector.memset(eps_sb[:], EPS)

            # persistent activations (live through the convs)
            x1T = persist.tile([128, NB, DC, SP], BF16, tag="x1T")
            x1n = persist.tile([128, NB, ST, D], F32, tag="x1n")
            for b in range(NB):
                nc.gpsimd.memset(x1T[:, b, :, 0:4], 0.0)
                nc.gpsimd.memset(x1T[:, b, :, 4 + S : SP], 0.0)

            for b in range(NB):
                qt, kt, vv = [], [], []
                for h in range(H):
                    qt.append(attn.tile([128, 2, S], BF16, name=f"qt{b}{h}", tag=f"qt{b}{h}"))
                    kt.append(attn.tile([128, 2, S], BF16, name=f"kt{b}{h}", tag=f"kt{b}{h}"))
                    vv.append(attn.tile([128, ST, DH + 1], BF16, name=f"vv{b}{h}", tag=f"vv{b}{h}"))
                    for wsb, bi, dst in ((wq_sb, 0, qt[h]), (wk_sb, 1, kt[h])):
                        for mc, (m0, msz) in enumerate(((0, 128), (128, 64))):
                            for qc in range(SQ):
                                ps = psA.tile([128, 512], F32, tag="p512")
                                for dc in range(DC):
                                    nc.tensor.matmul(
                                        ps[:msz, :],
                                        lhsT=wsb[:, h, dc, m0 : m0 + msz],
                                        rhs=xT_sb[b][:, dc, qc * 512 : qc * 512 + 512],
                                        start=(dc == 0),
                                        stop=(dc == DC - 1),
                                    )
                                nc.scalar.activation(
                                    out=dst[:msz, mc, qc * 512 : qc * 512 + 512],
                                    in_=ps[:msz, :],
                                    func=AF.Identity,
                                    bias=bqk_sb[:msz, bi, h, mc : mc + 1],
                                    scale=1.0,
                                )
                    for st in range(ST):
                        ps = psA.tile([128, 512], F32, tag="p512")
                        for dc in range(DC):
                            nc.tensor.matmul(
                                ps[:, :DH],
                                lhsT=xT_sb[b][:, dc, st * 128 : st * 128 + 128],
                                rhs=wv_sb[:, h, dc, :],
                                start=(dc == 0),
                                stop=(dc == DC - 1),
                            )
                        nc.vector.tensor_add(
                            out=vv[h][:, st, 0:DH], in0=ps[:, :DH], in1=bv_sb[:, h, :]
                        )
                        nc.vector.memset(vv[h][:, st, DH : DH + 1], 1.0)

                onrm = []
                for h in range(H):
                    expT = expp.tile([128, ST, S], BF16, name="expT", tag="expT")
                    for kc in range(ST):
                        for qc in range(SQ):
                            qs = slice(qc * 512, qc * 512 + 512)
                            ps = psB.tile([128, 512], F32, tag="sc")
                            for mc, (m0, msz) in enumerate(((0, 128), (128, 64))):
                                nc.tensor.matmul(
                                    ps[:],
                                    lhsT=kt[h][:msz, mc, kc * 128 : kc * 128 + 128],
                                    rhs=qt[h][:msz, mc, qs],
                                    start=(mc == 0),
                                    stop=(mc == 1),
                                )
                            mtile = mskp.tile([128, 512], U8, name="mt", tag="mt")
                            nc.sync.dma_start(mtile[:], mT_d[b, kc][:, qs])
                            nc.vector.scalar_tensor_tensor(
                                out=ps[:],
                                in0=mtile[:],
                                scalar=NEG,
                                in1=ps[:],
                                op0=ALU.mult,
                                op1=ALU.add,
                            )
                            nc.scalar.activation(
                                out=expT[:, kc, qs], in_=ps[:], func=AF.Exp,
                                scale=ISCALE,
                            )
                    onrm.append(attn.tile([128, 2, S], BF16, name=f"on{b}{h}", tag=f"on{b}{h}"))
                    for qc in range(SQ):
                        qs = slice(qc * 512, qc * 512 + 512)
                        ps0 = psC.tile([128, 512], F32, tag="ot0")
                        ps1 = psC.tile([65, 512], F32, tag="ot1")
                        for kc in range(ST):
                            nc.tensor.matmul(
                                ps0[:],
                                lhsT=vv[h][:, kc, 0:128],
                                rhs=expT[:, kc, qs],
                                start=(kc == 0),
                                stop=(kc == ST - 1),
                            )
                            nc.tensor.matmul(
                                ps1[:],
                                lhsT=vv[h][:, kc, 128 : DH + 1],
                                rhs=expT[:, kc, qs],
                                start=(kc == 0),
                                stop=(kc == ST - 1),
                            )
                        rc = smal.tile([1, 512], F32, tag="rc")
                        nc.vector.reciprocal(rc[:], ps1[64:65, :])
                        rb = smal.tile([128, 512], F32, tag="rb")
                        nc.gpsimd.partition_broadcast(rb[:], rc[:])
                        nc.vector.tensor_mul(out=onrm[h][:, 0, qs], in0=ps0[:], in1=rb[:])
                        nc.vector.tensor_mul(
                            out=onrm[h][:64, 1, qs], in0=ps1[:64, :], in1=rb[:64, :]
                        )

                # out-projection + residual + LN1 + transpose to x1T
                chunks = ((0, 0, 128, 0), (0, 1, 64, 1), (1, 0, 128, 2), (1, 1, 64, 3))
                for st in range(ST):
                    ps = psD.tile([128, D], F32, tag="at")
                    for i, (h, c, ksz, wc) in enumerate(chunks):
                        nc.tensor.matmul(
                            ps[:],
                            lhsT=onrm[h][:ksz, c, st * 128 : st * 128 + 128],
                            rhs=wo_sb[:ksz, wc, :],
                            start=(i == 0),
                            stop=(i == 3),
                        )
                    xn_t = lnp.tile([128, D], F32, tag="xn")
                    nc.sync.dma_start(xn_t[:], xn_d[b, st])
                    t = lnp.tile([128, D], F32, tag="t")
                    nc.vector.tensor_add(out=t[:], in0=ps[:], in1=xn_t[:])
                    nc.vector.tensor_add(out=t[:], in0=t[:], in1=bo_sb[:])
                    stats = lnp.tile([128, 6], F32, tag="st")
                    nc.vector.bn_stats(out=stats[:], in_=t[:])
                    mv = lnp.tile([128, 2], F32, tag="mv")
                    nc.vector.bn_aggr(out=mv[:], in_=stats[:])
                    sd = lnp.tile([128, 1], F32, tag="sd")
                    nc.scalar.activation(
                        out=sd[:], in_=mv[:, 1:2], func=AF.Sqrt, bias=eps_sb[:],
                    )
                    nc.vector.reciprocal(sd[:], sd[:])
                    xv = x1n[:, b, st, :]
                    nc.vector.tensor_scalar(
                        out=xv, in0=t[:], scalar1=mv[:, 0:1], scalar2=sd[:],
                        op0=ALU.subtract, op1=ALU.mult,
                    )
                    nc.vector.tensor_mul(out=xv, in0=xv, in1=g1_sb[:])
                    nc.vector.tensor_add(out=xv, in0=xv, in1=be1_sb[:])
                    for dc in range(DC):
                        tp = psE.tile([128, 128], F32, tag="tp")
                        nc.tensor.transpose(
                            tp[:], x1n[:, b, st, dc * 128 : dc * 128 + 128], ident[:]
                        )
                        nc.vector.tensor_copy(
                            out=x1T[:, b, dc, 4 + st * 128 : 4 + st * 128 + 128],
                            in_=tp[:],
                        )

        # ---- conv1 (+ prefetch of conv2 weights) ----
        with ExitStack() as cctx:
            w2p = cctx.enter_context(tc.tile_pool(name="w2p", bufs=1))
            w2 = w2p.tile([128, K, FT, D], BF16, tag="w2")
            nc.sync.dma_start(w2[:], wc2_d[:].rearrange("k c p e -> p k c e"))

            hTp = cctx.enter_context(tc.tile_pool(name="hTp", bufs=1))
            hT = hTp.tile([128, NB, FT, SP], BF16, tag="hT")
            for b in range(NB):
                nc.gpsimd.memset(hT[:, b, :, 0:4], 0.0)
                nc.gpsimd.memset(hT[:, b, :, 4 + S : SP], 0.0)

            with (
                tc.tile_pool(name="w1p", bufs=2) as w1p,
                tc.tile_pool(name="psF", bufs=4, space="PSUM") as psF,
            ):
                for ft in range(FT):
                    w1 = w1p.tile([128, K, DC, 128], BF16, tag="w1")
                    nc.sync.dma_start(
                        w1[:],
                        wc1_d[:, :, :, ft * 128 : ft * 128 + 128].rearrange(
                            "k c p f -> p k c f"
                        ),
                    )
                    for b in range(NB):
                        for qc in range(SQ):
                            ps = psF.tile([128, 512], F32, tag="c1")
                            idx = 0
                            for k9 in range(K):
                                for dc in range(DC):
                                    nc.tensor.matmul(
                                        ps[:],
                                        lhsT=w1[:, k9, dc, :],
                                        rhs=x1T[:, b, dc, qc * 512 + k9 : qc * 512 + k9 + 512],
                                        start=(idx == 0),
                                        stop=(idx == K * DC - 1),
                                    )
                                    idx += 1
                            nc.scalar.activation(
                                out=hT[:, b, ft, 4 + qc * 512 : 4 + qc * 512 + 512],
                                in_=ps[:],
                                func=AF.Relu,
                                bias=bc1_sb[:, ft : ft + 1],
                                scale=1.0,
                            )

            # ---- conv2 + residual + LN2 -> y ----
            with (
                tc.tile_pool(name="psG", bufs=4, space="PSUM") as psG,
                tc.tile_pool(name="ln2", bufs=3) as ln2,
            ):
                for b in range(NB):
                    for st in range(ST):
                        ps = psG.tile([128, D], F32, tag="c2")
                        idx = 0
                        for k9 in range(K):
                            for fc in range(FT):
                                nc.tensor.matmul(
                                    ps[:],
                                    lhsT=hT[:, b, fc, st * 128 + k9 : st * 128 + k9 + 128],
                                    rhs=w2[:, k9, fc, :],
                                    start=(idx == 0),
                                    stop=(idx == K * FT - 1),
                                )
                                idx += 1
                        t = ln2.tile([128, D], F32, tag="t")
                        nc.vector.tensor_add(out=t[:], in0=ps[:], in1=x1n[:, b, st, :])
                        nc.vector.tensor_add(out=t[:], in0=t[:], in1=bc2_sb[:])
                        stats = ln2.tile([128, 6], F32, tag="st")
                        nc.vector.bn_stats(out=stats[:], in_=t[:])
                        mv = ln2.tile([128, 2], F32, tag="mv")
                        nc.vector.bn_aggr(out=mv[:], in_=stats[:])
                        sd = ln2.tile([128, 1], F32, tag="sd")
                        nc.scalar.activation(
                            out=sd[:], in_=mv[:, 1:2], func=AF.Sqrt, bias=eps_sb[:],
                        )
                        nc.vector.reciprocal(sd[:], sd[:])
                        ot = ln2.tile([128, D], F32, tag="o")
                        nc.vector.tensor_scalar(
                            out=ot[:], in0=t[:], scalar1=mv[:, 0:1], scalar2=sd[:],
                            op0=ALU.subtract, op1=ALU.mult,
                        )
                        nc.vector.tensor_mul(out=ot[:], in0=ot[:], in1=g2_sb[:])
                        nc.vector.tensor_add(out=ot[:], in0=ot[:], in1=be2_sb[:])
                        nc.sync.dma_start(y_d[b, st], ot[:])


def _build():
    if "nc" not in _CACHE:
        nc = bacc.Bacc()
        _CACHE["nc"] = _emit(nc)
    return _CACHE["nc"]


def _prep_shared(Wq, bq, Wk, bk, Wv, bv, Wo, bo, Wc1, bc1, Wc2, bc2, g1, beta1, g2, beta2):
    bf = ml_dtypes.bfloat16
    f32 = np.float32
    sh = {}
    sh["wq"] = np.ascontiguousarray(Wq.reshape(H, DC, 128, DH).astype(bf))
    sh["wk"] = np.ascontiguousarray(Wk.reshape(H, DC, 128, DH).astype(bf))
    sh["wv"] = np.ascontiguousarray(Wv.reshape(H, DC, 128, DH).astype(bf))
    wo = np.zeros((4, 128, D), dtype=bf)
    bounds = ((0, 128), (128, 192), (192, 320), (320, 384))
    for c, (r0, r1) in enumerate(bounds):
        wo[c, : r1 - r0] = Wo[r0:r1].astype(bf)
    sh["wo"] = wo
    sh["wc1"] = np.ascontiguousarray(Wc1.reshape(K, DC, 128, F).astype(bf))
    sh["wc2"] = np.ascontiguousarray(Wc2.reshape(K, FT, 128, D).astype(bf))
    bqk = np.zeros((2, H, 2, 128), dtype=f32)
    for i, bb in enumerate((bq, bk)):
        for h in range(H):
            bqk[i, h, 0, :] = bb[h, :128]
            bqk[i, h, 1, :64] = bb[h, 128:]
    sh["bqk"] = bqk
    sh["bv"] = bv.astype(f32)
    sh["bo"] = bo.astype(f32)
    sh["bc1t"] = np.ascontiguousarray(bc1.reshape(FT, 128).T.astype(f32))
    sh["bc2"] = bc2.astype(f32)
    sh["g1"] = g1.astype(f32)
    sh["be1"] = beta1.astype(f32)
    sh["g2"] = g2.astype(f32)
    sh["be2"] = beta2.astype(f32)
    return sh


def run_sharded(inputs, trace=False):
    nc = _build()
    x = np.asarray(inputs["x"], dtype=np.float32)
    mask = np.asarray(inputs["mask"])
    sh = _prep_shared(
        *[np.asarray(inputs[k]) for k in (
            "Wq", "bq", "Wk", "bk", "Wv", "bv", "Wo", "bo",
            "Wc1", "bc1", "Wc2", "bc2", "g1", "beta1", "g2", "beta2",
        )]
    )
    bf = ml_dtypes.bfloat16
    in_maps = []
    for c in range(NCORES):
        xb = x[c * NB : (c + 1) * NB]  # [NB, S, D]
        m = {}
        m["xT"] = np.ascontiguousarray(xb.transpose(0, 2, 1)).reshape(NB, DC, 128, S).astype(bf)
        m["xn"] = np.ascontiguousarray(xb.reshape(NB, ST, 128, D))
        mb = mask[c * NB : (c + 1) * NB]
        m["mT"] = np.ascontiguousarray(mb.transpose(0, 2, 1)).astype(np.uint8).reshape(NB, ST, 128, S)
        m.update(sh)
        in_maps.append(m)
    res = run_bass_kernel_spmd(nc, in_maps, core_ids=list(range(NCORES)), trace=trace)
    out = np.empty((B, S, D), dtype=np.float32)
    for c in range(NCORES):
        out[c * NB : (c + 1) * NB] = res.results[c]["y"].reshape(NB, S, D)
    return out, res


def kernel(**inputs):
    out, _ = run_sharded(inputs, trace=False)
    return out
